# revision 10
# baseline (speedup 1.0000x reference)
"""MoE routing kernel for Trainium2, 8-core expert-parallel.

Strategy: each core owns 2 of 16 experts (expert-parallel, per the sharding
hint). Gating (fp32 matmul + top-4 softmax) is replicated on every core —
it is tiny — so the load-balance loss needs no collective. Each core runs
its two experts' MLPs densely over all 1024 tokens with float32r matmuls
(full-rate fp32 storage, TF32-ish compute), folds the gate-weighted combine
and the modality sum into PE matmuls, then an AllReduce of the [64, 1024]
partial combines expert groups. LayerNorm + LOS head + both aux losses are
replicated on-device; the host just shards inputs and reads core 0's output.
"""

import os

import numpy as np

import concourse.mybir as mybir
import concourse.tile as tile
from concourse import bacc
from concourse import bass_utils

N_CORES = 8
E, K = 16, 4
D, H = 1024, 1024
B, M = 64, 16
N = B * M          # 1024 tokens
OUT = 10
TG = 512           # token group (matmul moving free dim)
NG = N // TG       # 2 token groups
DC = D // 128      # 8
HC = H // 128      # 8
TC = N // 128      # 8 token tiles
EPC = E // N_CORES  # 2 experts per core

_CACHE: dict = {}


def _build(n_cores=N_CORES, skip_cc=False):
    dt = mybir.dt
    nc = bacc.Bacc("TRN2", target_bir_lowering=False, debug=False,
                   num_devices=n_cores)

    # ---- DRAM I/O ----
    xtr_d = nc.dram_tensor("xtr", [D, N], dt.float32r, kind="ExternalInput").ap()
    wg_d = nc.dram_tensor("wg", [D, E], dt.float32, kind="ExternalInput").ap()
    w1_d = nc.dram_tensor("w1c", [EPC, D, H], dt.float32r, kind="ExternalInput").ap()
    w2_d = nc.dram_tensor("w2c", [EPC, H, D], dt.float32r, kind="ExternalInput").ap()
    b1_d = nc.dram_tensor("b1r", [EPC, 128, HC], dt.float32, kind="ExternalInput").ap()
    b2_d = nc.dram_tensor("b2c", [EPC, D], dt.float32, kind="ExternalInput").ap()
    smat_d = nc.dram_tensor("smat", [N, B], dt.float32, kind="ExternalInput").ap()
    esel_d = nc.dram_tensor("esel", [EPC, 128, E], dt.float32, kind="ExternalInput").ap()
    eselt_d = nc.dram_tensor("eselt", [E, EPC], dt.float32, kind="ExternalInput").ap()
    lnw_d = nc.dram_tensor("lnw_b", [B, D], dt.float32, kind="ExternalInput").ap()
    lnb_d = nc.dram_tensor("lnb_b", [B, D], dt.float32, kind="ExternalInput").ap()
    wlos_d = nc.dram_tensor("wlos", [D, OUT], dt.float32, kind="ExternalInput").ap()
    blos_d = nc.dram_tensor("blos", [OUT, 1], dt.float32, kind="ExternalInput").ap()
    ty_d = nc.dram_tensor("ty", [B, OUT], dt.float32, kind="ExternalInput").ap()
    ident_d = nc.dram_tensor("ident", [128, 128], dt.float32, kind="ExternalInput").ap()

    scores_d = nc.dram_tensor("scores", [B, OUT], dt.float32, kind="ExternalOutput").ap()
    loss_d = nc.dram_tensor("loss", [1, 1], dt.float32, kind="ExternalOutput").ap()
    ploss_d = nc.dram_tensor("ploss", [1, 1], dt.float32, kind="ExternalOutput").ap()

    f32, f32r = dt.float32, dt.float32r
    AF = mybir.ActivationFunctionType
    AX = mybir.AxisListType

    with tile.TileContext(nc) as tc:
        with tc.tile_pool(name="const", bufs=1) as cpool, \
             tc.tile_pool(name="xtr", bufs=DC) as xpool, \
             tc.tile_pool(name="w1", bufs=12) as w1pool, \
             tc.tile_pool(name="w2", bufs=12) as w2pool, \
             tc.tile_pool(name="hT", bufs=9) as hpool, \
             tc.tile_pool(name="osb", bufs=3) as opool, \
             tc.tile_pool(name="sm", bufs=1) as spool, \
             tc.tile_pool(name="mm", bufs=2, space="PSUM") as mmps, \
             tc.tile_pool(name="big", bufs=3, space="PSUM") as bigps, \
             tc.tile_pool(name="small", bufs=2, space="PSUM") as smps, \
             tc.tile_pool(name="dram", bufs=2, space="DRAM") as dpool:

            # ---- constant/small loads (issued first so gating can start) ----
            xtr = []
            for dc in range(DC):
                t = xpool.tile([128, N], f32r)
                nc.sync.dma_start(t[:], xtr_d[dc * 128:(dc + 1) * 128, :])
                xtr.append(t)
            wg_t = cpool.tile([128, DC * E], f32)
            for dc in range(DC):
                nc.sync.dma_start(
                    wg_t[:, dc * E:(dc + 1) * E], wg_d[dc * 128:(dc + 1) * 128, :])
            ident = cpool.tile([128, 128], f32)
            nc.sync.dma_start(ident[:], ident_d[:])
            smat_t = []
            for t_ in range(TC):
                s = cpool.tile([128, B], f32, tag="smat", bufs=TC)
                nc.sync.dma_start(s[:], smat_d[t_ * 128:(t_ + 1) * 128, :])
                smat_t.append(s)
            esel_t = []
            for e in range(EPC):
                s = cpool.tile([128, E], f32, tag="esel", bufs=EPC)
                nc.sync.dma_start(s[:], esel_d[e])
                esel_t.append(s)
            eselt_t = cpool.tile([E, EPC], f32)
            nc.sync.dma_start(eselt_t[:], eselt_d[:])
            b1_t = []
            for e in range(EPC):
                s = cpool.tile([128, HC], f32, tag="b1", bufs=EPC)
                nc.sync.dma_start(s[:], b1_d[e])
                b1_t.append(s)
            b2_t = cpool.tile([EPC, D], f32)
            nc.sync.dma_start(b2_t[:], b2_d[:])
            lnw_t = cpool.tile([B, D], f32)
            nc.sync.dma_start(lnw_t[:], lnw_d[:])
            lnb_t = cpool.tile([B, D], f32)
            nc.sync.dma_start(lnb_t[:], lnb_d[:])
            wlos_t = cpool.tile([128, DC * OUT], f32)
            for dc in range(DC):
                nc.sync.dma_start(
                    wlos_t[:, dc * OUT:(dc + 1) * OUT],
                    wlos_d[dc * 128:(dc + 1) * 128, :])
            blos_t = cpool.tile([OUT, 1], f32)
            nc.sync.dma_start(blos_t[:], blos_d[:])
            ty_t = cpool.tile([B, OUT], f32)
            nc.sync.dma_start(ty_t[:], ty_d[:])
            ones = cpool.tile([128, 1], f32)
            nc.vector.memset(ones[:], 1.0)

            # ---- expert weights (streamed; Tile prefetches into free slots) ----
            w1_t = [[None] * DC for _ in range(EPC)]
            w2_t = [[None] * HC for _ in range(EPC)]
            for e in range(EPC):
                for dc in range(DC):
                    t = w1pool.tile([128, H], f32r, tag="w1")
                    nc.sync.dma_start(t[:], w1_d[e, dc * 128:(dc + 1) * 128, :])
                    w1_t[e][dc] = t
                for hc in range(HC):
                    t = w2pool.tile([128, D], f32r, tag="w2")
                    nc.sync.dma_start(t[:], w2_d[e, hc * 128:(hc + 1) * 128, :])
                    w2_t[e][hc] = t

            # ---- gating: logitsT = wg.T @ x (fp32), transpose, top-4 softmax ----
            logits_t = []
            for g in range(NG):
                lgt_ps = smps.tile([E, TG], f32, tag="sp")
                for dc in range(DC):
                    nc.tensor.matmul(
                        lgt_ps[:],
                        wg_t[:, dc * E:(dc + 1) * E],
                        xtr[dc][:, g * TG:(g + 1) * TG].bitcast(f32),
                        start=(dc == 0), stop=(dc == DC - 1))
                lgt_sb = spool.tile([E, TG], f32, tag="lgt")
                nc.scalar.copy(lgt_sb[:], lgt_ps[:])
                for c8 in range(TG // 128):
                    tp = smps.tile([128, E], f32, tag="sp")
                    nc.tensor.transpose(
                        tp[:], lgt_sb[:, c8 * 128:(c8 + 1) * 128], ident[:E, :E])
                    lt = spool.tile([128, E], f32, tag="logits", bufs=8)
                    nc.scalar.copy(lt[:], tp[:])
                    logits_t.append(lt)

            gates_t, mask_t, gcol = [], [], []
            for t_ in range(TC):
                lt = logits_t[t_]
                mx8 = spool.tile([128, 8], f32, tag="mx8", bufs=2)
                nc.vector.max(mx8[:], lt[:])
                negm = spool.tile([128, 1], f32, tag="negm", bufs=2)
                nc.vector.reduce_max(negm[:], lt[:], axis=AX.X, negate=True)
                mask = spool.tile([128, E], f32, tag="mask", bufs=8)
                nc.vector.tensor_scalar(
                    mask[:], lt[:], mx8[:, 3:4], None, op0=mybir.AluOpType.is_ge)
                el = spool.tile([128, E], f32, tag="el", bufs=2)
                nc.scalar.activation(el[:], lt[:], AF.Exp, bias=negm[:, 0:1])
                gated = spool.tile([128, E], f32, tag="gated", bufs=2)
                nc.vector.tensor_mul(gated[:], el[:], mask[:])
                denom = spool.tile([128, 1], f32, tag="denom", bufs=2)
                nc.vector.reduce_sum(denom[:], gated[:], axis=AX.X)
                rden = spool.tile([128, 1], f32, tag="rden", bufs=2)
                nc.vector.reciprocal(rden[:], denom[:])
                gates = spool.tile([128, E], f32, tag="gates", bufs=8)
                nc.vector.tensor_scalar_mul(gates[:], gated[:], rden[:, 0:1])
                gates_t.append(gates)
                mask_t.append(mask)
                # per-expert gate columns for this core (data-driven select)
                cols = []
                for e in range(EPC):
                    gm = spool.tile([128, E], f32, tag="gm", bufs=2)
                    nc.vector.tensor_mul(gm[:], gates[:], esel_t[e][:])
                    gc = spool.tile([128, 1], f32, tag="gcol", bufs=16)
                    nc.vector.reduce_sum(gc[:], gm[:], axis=AX.X)
                    cols.append(gc)
                gcol.append(cols)

            # ---- aux loss: importance / load ----
            def colsum16(tiles, tag):
                ps = smps.tile([1, E], f32, tag="sp")
                for t_ in range(TC):
                    nc.tensor.matmul(ps[:], ones[:], tiles[t_][:],
                                     start=(t_ == 0), stop=(t_ == TC - 1))
                sb = spool.tile([1, E], f32, tag=tag)
                nc.scalar.copy(sb[:], ps[:])
                return sb

            imp_sb = colsum16(gates_t, "imp")
            load_sb = colsum16(mask_t, "load")

            def cv_parts(v16, tag):
                # returns (sum_sq_dev [1,1], recip_mean2e [1,1]); cv = ssd/15 * r
                s = spool.tile([1, 1], f32, tag=tag + "s")
                nc.vector.reduce_sum(s[:], v16[:], axis=AX.X)
                mean = spool.tile([1, 1], f32, tag=tag + "m")
                nc.vector.tensor_scalar_mul(mean[:], s[:], 1.0 / E)
                d = spool.tile([1, E], f32, tag=tag + "d")
                nc.vector.tensor_scalar(
                    d[:], v16[:], mean[0:1, 0:1], None,
                    op0=mybir.AluOpType.subtract)
                d2 = spool.tile([1, E], f32, tag=tag + "d2")
                nc.vector.tensor_mul(d2[:], d[:], d[:])
                ssd = spool.tile([1, 1], f32, tag=tag + "v")
                nc.vector.reduce_sum(ssd[:], d2[:], axis=AX.X)
                m2 = spool.tile([1, 1], f32, tag=tag + "m2")
                nc.vector.tensor_mul(m2[:], mean[:], mean[:])
                m2e = spool.tile([1, 1], f32, tag=tag + "m2e")
                nc.vector.tensor_scalar_add(m2e[:], m2[:], 1e-10)
                r = spool.tile([1, 1], f32, tag=tag + "r")
                nc.vector.reciprocal(r[:], m2e[:])
                cv = spool.tile([1, 1], f32, tag=tag + "cv")
                nc.vector.tensor_mul(cv[:], ssd[:], r[:])
                return cv

            cvi = cv_parts(imp_sb, "ci")
            cvl = cv_parts(load_sb, "cl")
            cvs = spool.tile([1, 1], f32, tag="cvs")
            nc.vector.tensor_add(cvs[:], cvi[:], cvl[:])
            loss_sb = spool.tile([1, 1], f32, tag="lossv")
            nc.vector.tensor_scalar_mul(loss_sb[:], cvs[:], 0.01 / (E - 1))
            nc.sync.dma_start(loss_d[:], loss_sb[:])

            # ---- gb = S.T @ gates (per-batch gate sums), then this core's rows ----
            gb_ps = smps.tile([B, E], f32, tag="sp")
            for t_ in range(TC):
                nc.tensor.matmul(gb_ps[:], smat_t[t_][:], gates_t[t_][:],
                                 start=(t_ == 0), stop=(t_ == TC - 1))
            gb_sb = spool.tile([B, E], f32, tag="gb")
            nc.scalar.copy(gb_sb[:], gb_ps[:])
            gbt_ps = smps.tile([E, B], f32, tag="sp")
            nc.tensor.transpose(gbt_ps[:], gb_sb[:], ident[:B, :B])
            gbt_sb = spool.tile([E, B], f32, tag="gbt")
            nc.scalar.copy(gbt_sb[:], gbt_ps[:])
            gbt2_ps = smps.tile([EPC, B], f32, tag="sp")
            nc.tensor.matmul(gbt2_ps[:], eselt_t[:], gbt_sb[:], start=True, stop=True)
            gbt2_sb = spool.tile([EPC, B], f32, tag="gbt2")
            nc.scalar.copy(gbt2_sb[:], gbt2_ps[:])

            # ---- expert MLPs + gated combine + modality sum (into mm psum) ----
            SKIP = os.environ.get("KSKIP", "")
            mm_ps = [mmps.tile([B, TG], f32, tag="mm", name=f"mm_ps{i}")
                     for i in range(2)]
            first_mm = [True, True]
            for e in range(0 if "experts" in SKIP else EPC):
                for g in range(NG):
                    hT = []
                    for hc in range(HC):
                        ph = bigps.tile([128, TG], f32, tag="big")
                        for dc in range(DC):
                            nc.tensor.matmul(
                                ph[:],
                                w1_t[e][dc][:, hc * 128:(hc + 1) * 128],
                                xtr[dc][:, g * TG:(g + 1) * TG],
                                start=(dc == 0), stop=(dc == DC - 1))
                        ht = hpool.tile([128, TG], f32r, tag="hT")
                        nc.scalar.activation(ht[:], ph[:], AF.Relu,
                                             bias=b1_t[e][:, hc:hc + 1])
                        hT.append(ht)
                    for sub in range(TG // 128):
                        tglob = g * (TG // 128) + sub
                        ge = spool.tile([128, B], f32r, tag="ge", bufs=3)
                        nc.vector.tensor_scalar_mul(
                            ge[:], smat_t[tglob][:], gcol[tglob][e][:, 0:1])
                        for dh in range(2):
                            po = bigps.tile([128, TG], f32, tag="big")
                            for hc in range(HC):
                                nc.tensor.matmul(
                                    po[:],
                                    hT[hc][:, sub * 128:(sub + 1) * 128],
                                    w2_t[e][hc][:, dh * TG:(dh + 1) * TG],
                                    start=(hc == 0), stop=(hc == HC - 1))
                            osb = opool.tile([128, TG], f32r, tag="osb")
                            nc.scalar.copy(osb[:], po[:])
                            nc.tensor.matmul(mm_ps[dh][:], ge[:], osb[:],
                                             start=first_mm[dh], stop=False)
                            first_mm[dh] = False
            # b2 contribution: gbt2.T @ b2c rows (K=2), closes the mm groups
            for dh in range(2):
                nc.tensor.matmul(mm_ps[dh][:], gbt2_sb[:],
                                 b2_t[:, dh * TG:(dh + 1) * TG].bitcast(f32),
                                 start=("experts" in SKIP), stop=True)

            # ---- AllReduce partial mm across expert groups ----
            mm_sb = spool.tile([B, D], f32, tag="lnbuf", bufs=3)
            for dh in range(2):
                nc.scalar.copy(mm_sb[:, dh * TG:(dh + 1) * TG], mm_ps[dh][:])
            in_b = dpool.tile([B, D], f32)
            out_b = dpool.tile([B, D], f32, addr_space="Shared")
            nc.sync.dma_start(in_b[:], mm_sb[:])
            if skip_cc:
                nc.sync.dma_start(out_b[:], in_b[:])
            else:
                nc.gpsimd.collective_compute(
                    "AllReduce", mybir.AluOpType.add,
                    replica_groups=[list(range(n_cores))],
                    ins=[in_b.opt()], outs=[out_b.opt()])
            mmr = spool.tile([B, D], f32, tag="lnbuf", bufs=3)
            nc.sync.dma_start(mmr[:], out_b[:])

            # ---- LayerNorm over D ----
            s1 = spool.tile([B, 1], f32, tag="s1")
            nc.vector.reduce_sum(s1[:], mmr[:], axis=AX.X)
            negmu = spool.tile([B, 1], f32, tag="negmu")
            nc.vector.tensor_scalar_mul(negmu[:], s1[:], -1.0 / D)
            xc = spool.tile([B, D], f32, tag="lnbuf", bufs=3)
            nc.vector.tensor_scalar_add(xc[:], mmr[:], negmu[:, 0:1])
            x2 = spool.tile([B, D], f32, tag="lnbuf", bufs=3)
            nc.vector.tensor_mul(x2[:], xc[:], xc[:])
            s2 = spool.tile([B, 1], f32, tag="s2")
            nc.vector.reduce_sum(s2[:], x2[:], axis=AX.X)
            epsb = spool.tile([B, 1], f32, tag="epsb")
            nc.vector.memset(epsb[:], 1e-5)
            std = spool.tile([B, 1], f32, tag="std")
            nc.scalar.activation(std[:], s2[:], AF.Sqrt, bias=epsb[:, 0:1],
                                 scale=1.0 / D)
            rstd = spool.tile([B, 1], f32, tag="rstd")
            nc.vector.reciprocal(rstd[:], std[:])
            xn = spool.tile([B, D], f32, tag="lnbuf", bufs=3)
            nc.vector.tensor_scalar_mul(xn[:], xc[:], rstd[:, 0:1])
            xw = spool.tile([B, D], f32, tag="lnbuf", bufs=3)
            nc.vector.tensor_mul(xw[:], xn[:], lnw_t[:])
            fin = spool.tile([B, D], f32, tag="lnbuf", bufs=3)
            nc.vector.tensor_add(fin[:], xw[:], lnb_t[:])

            # ---- LOS head: scoresT = wlos.T @ finT, + b_los ----
            st_ps = smps.tile([OUT, B], f32, tag="sp")
            for dc in range(DC):
                tp = smps.tile([128, B], f32, tag="sp")
                nc.tensor.transpose(
                    tp[:], fin[:, dc * 128:(dc + 1) * 128], ident[:B, :B])
                ft = spool.tile([128, B], f32, tag="fT", bufs=2)
                nc.scalar.copy(ft[:], tp[:])
                nc.tensor.matmul(st_ps[:], wlos_t[:, dc * OUT:(dc + 1) * OUT],
                                 ft[:], start=(dc == 0), stop=(dc == DC - 1))
            st_sb = spool.tile([OUT, B], f32, tag="stsb")
            nc.scalar.add(st_sb[:], st_ps[:], blos_t[:, 0:1])
            sc_ps = smps.tile([B, OUT], f32, tag="sp")
            nc.tensor.transpose(sc_ps[:], st_sb[:], ident[:OUT, :OUT])
            sc_sb = spool.tile([B, OUT], f32, tag="scsb")
            nc.scalar.copy(sc_sb[:], sc_ps[:])
            nc.sync.dma_start(scores_d[:], sc_sb[:])

            # ---- pred_loss = mean((scores - y)^2) ----
            df = spool.tile([B, OUT], f32, tag="df")
            nc.vector.tensor_sub(df[:], sc_sb[:], ty_t[:])
            dfs = spool.tile([B, OUT], f32, tag="dfs")
            nc.vector.tensor_mul(dfs[:], df[:], df[:])
            rs = spool.tile([B, 1], f32, tag="rs")
            nc.vector.reduce_sum(rs[:], dfs[:], axis=AX.X)
            pl_ps = smps.tile([1, 1], f32, tag="sp")
            nc.tensor.matmul(pl_ps[:], ones[:B, :], rs[:], start=True, stop=True)
            pl_sb = spool.tile([1, 1], f32, tag="plsb")
            nc.scalar.mul(pl_sb[:], pl_ps[:], 1.0 / (B * OUT))
            nc.sync.dma_start(ploss_d[:], pl_sb[:])

    nc.compile()
    return nc


def _host_inputs(inputs):
    f = np.float32
    x = np.asarray(inputs["mm_embed"], f).reshape(N, D)
    xT = np.ascontiguousarray(x.T)
    wg = np.asarray(inputs["w_gate"], f)
    W1 = np.asarray(inputs["W1"], f)
    b1 = np.asarray(inputs["b1"], f)
    W2 = np.asarray(inputs["W2"], f)
    b2 = np.asarray(inputs["b2"], f)
    lnw_b = np.broadcast_to(np.asarray(inputs["ln_w"], f), (B, D)).copy()
    lnb_b = np.broadcast_to(np.asarray(inputs["ln_b"], f), (B, D)).copy()
    wlos = np.asarray(inputs["W_los"], f)
    blos = np.asarray(inputs["b_los"], f).reshape(OUT, 1)
    ty = np.asarray(inputs["true_y"], f)
    smat = np.zeros((N, B), f)
    smat[np.arange(N), np.arange(N) // M] = 1.0
    ident = np.eye(128, dtype=f)

    in_maps = []
    for c in range(N_CORES):
        es = np.zeros((EPC, 128, E), f)
        est = np.zeros((E, EPC), f)
        for e in range(EPC):
            es[e, :, c * EPC + e] = 1.0
            est[c * EPC + e, e] = 1.0
        b1r = np.ascontiguousarray(
            b1[c * EPC:(c + 1) * EPC].reshape(EPC, HC, 128).transpose(0, 2, 1))
        in_maps.append({
            "xtr": xT,
            "wg": wg,
            "w1c": np.ascontiguousarray(W1[c * EPC:(c + 1) * EPC]),
            "w2c": np.ascontiguousarray(W2[c * EPC:(c + 1) * EPC]),
            "b1r": b1r,
            "b2c": np.ascontiguousarray(b2[c * EPC:(c + 1) * EPC]),
            "smat": smat,
            "esel": es,
            "eselt": est,
            "lnw_b": lnw_b,
            "lnb_b": lnb_b,
            "wlos": wlos,
            "blos": blos,
            "ty": ty,
            "ident": ident,
        })
    return in_maps


def get_nc():
    if "nc" not in _CACHE:
        _CACHE["nc"] = _build()
    return _CACHE["nc"]


def kernel(**inputs):
    nc = get_nc()
    in_maps = _host_inputs(inputs)
    res = bass_utils.run_bass_kernel_spmd(nc, in_maps, core_ids=list(range(N_CORES)))
    r0 = res.results[0]
    scores = np.asarray(r0["scores"], np.float32)
    loss = np.asarray(r0["loss"], np.float32).reshape(())
    ploss = np.asarray(r0["ploss"], np.float32).reshape(())
    return (scores, loss, ploss)


if __name__ == "__main__":
    import reference
    inputs = {k: np.asarray(v) if not np.isscalar(v) else v
              for k, v in reference.setup_inputs().items()}
    got = kernel(**inputs)
    exp = reference.reference(**reference.setup_inputs())
    for name, g_, e_ in zip(("scores", "loss", "pred_loss"), got, exp):
        e_ = np.asarray(e_)
        rel = np.abs(g_ - e_).max() / (np.abs(e_).max() + 1e-12)
        print(f"{name}: rel err {rel:.3e}")


# revision 11
# speedup vs baseline: 1.0369x; 1.0369x over previous
"""MoE routing kernel for Trainium2, 8-core expert-parallel.

Strategy: each core owns 2 of 16 experts (expert-parallel, per the sharding
hint). Gating (fp32 matmul + top-4 softmax) is replicated on every core —
it is tiny — so the load-balance loss needs no collective. Each core runs
its two experts' MLPs densely over all 1024 tokens with float32r matmuls
(full-rate fp32 storage, TF32-ish compute), folds the gate-weighted combine
and the modality sum into PE matmuls, then an AllReduce of the [64, 1024]
partial combines expert groups. LayerNorm + LOS head + both aux losses are
replicated on-device; the host just shards inputs and reads core 0's output.
"""

import os

import numpy as np

import concourse.mybir as mybir
import concourse.tile as tile
from concourse import bacc
from concourse import bass_utils

N_CORES = 8
E, K = 16, 4
D, H = 1024, 1024
B, M = 64, 16
N = B * M          # 1024 tokens
OUT = 10
TG = 512           # token group (matmul moving free dim)
NG = N // TG       # 2 token groups
DC = D // 128      # 8
HC = H // 128      # 8
TC = N // 128      # 8 token tiles
EPC = E // N_CORES  # 2 experts per core

_CACHE: dict = {}


def _build(n_cores=N_CORES, skip_cc=False):
    dt = mybir.dt
    nc = bacc.Bacc("TRN2", target_bir_lowering=False, debug=False,
                   num_devices=n_cores)

    # ---- DRAM I/O ----
    xtr_d = nc.dram_tensor("xtr", [D, N], dt.float32r, kind="ExternalInput").ap()
    wg_d = nc.dram_tensor("wg", [D, E], dt.float32, kind="ExternalInput").ap()
    w1_d = nc.dram_tensor("w1c", [EPC, D, H], dt.float32r, kind="ExternalInput").ap()
    w2_d = nc.dram_tensor("w2c", [EPC, H, D], dt.float32r, kind="ExternalInput").ap()
    b1_d = nc.dram_tensor("b1r", [EPC, 128, HC], dt.float32, kind="ExternalInput").ap()
    b2_d = nc.dram_tensor("b2c", [EPC, D], dt.float32, kind="ExternalInput").ap()
    smat_d = nc.dram_tensor("smat", [N, B], dt.float32, kind="ExternalInput").ap()
    esel_d = nc.dram_tensor("esel", [EPC, 128, E], dt.float32, kind="ExternalInput").ap()
    eselt_d = nc.dram_tensor("eselt", [E, EPC], dt.float32, kind="ExternalInput").ap()
    lnw_d = nc.dram_tensor("lnw_b", [B, D], dt.float32, kind="ExternalInput").ap()
    lnb_d = nc.dram_tensor("lnb_b", [B, D], dt.float32, kind="ExternalInput").ap()
    wlos_d = nc.dram_tensor("wlos", [D, OUT], dt.float32, kind="ExternalInput").ap()
    blos_d = nc.dram_tensor("blos", [OUT, 1], dt.float32, kind="ExternalInput").ap()
    ty_d = nc.dram_tensor("ty", [B, OUT], dt.float32, kind="ExternalInput").ap()
    ident_d = nc.dram_tensor("ident", [128, 128], dt.float32, kind="ExternalInput").ap()

    scores_d = nc.dram_tensor("scores", [B, OUT], dt.float32, kind="ExternalOutput").ap()
    loss_d = nc.dram_tensor("loss", [1, 1], dt.float32, kind="ExternalOutput").ap()
    ploss_d = nc.dram_tensor("ploss", [1, 1], dt.float32, kind="ExternalOutput").ap()

    f32, f32r = dt.float32, dt.float32r
    AF = mybir.ActivationFunctionType
    AX = mybir.AxisListType

    with tile.TileContext(nc) as tc:
        with tc.tile_pool(name="const", bufs=1) as cpool, \
             tc.tile_pool(name="xtr", bufs=DC) as xpool, \
             tc.tile_pool(name="w1", bufs=12) as w1pool, \
             tc.tile_pool(name="w2", bufs=12) as w2pool, \
             tc.tile_pool(name="hT", bufs=9) as hpool, \
             tc.tile_pool(name="osb", bufs=3) as opool, \
             tc.tile_pool(name="sm", bufs=1) as spool, \
             tc.tile_pool(name="mm", bufs=2, space="PSUM") as mmps, \
             tc.tile_pool(name="big", bufs=3, space="PSUM") as bigps, \
             tc.tile_pool(name="small", bufs=2, space="PSUM") as smps, \
             tc.tile_pool(name="dram", bufs=2, space="DRAM") as dpool:

            # ---- small consts first (gating needs wg before the bulk loads) ----
            wg_t = cpool.tile([128, DC * E], f32)
            for dc in range(DC):
                nc.sync.dma_start(
                    wg_t[:, dc * E:(dc + 1) * E], wg_d[dc * 128:(dc + 1) * 128, :])
            ident = cpool.tile([128, 128], f32)
            nc.sync.dma_start(ident[:], ident_d[:])
            xtr = []
            for dc in range(DC):
                t = xpool.tile([128, N], f32r)
                nc.sync.dma_start(t[:], xtr_d[dc * 128:(dc + 1) * 128, :])
                xtr.append(t)
            smat_t = []
            for t_ in range(TC):
                s = cpool.tile([128, B], f32, tag="smat", bufs=TC)
                nc.sync.dma_start(s[:], smat_d[t_ * 128:(t_ + 1) * 128, :])
                smat_t.append(s)
            esel_t = []
            for e in range(EPC):
                s = cpool.tile([128, E], f32, tag="esel", bufs=EPC)
                nc.sync.dma_start(s[:], esel_d[e])
                esel_t.append(s)
            eselt_t = cpool.tile([E, EPC], f32)
            nc.sync.dma_start(eselt_t[:], eselt_d[:])
            b1_t = []
            for e in range(EPC):
                s = cpool.tile([128, HC], f32, tag="b1", bufs=EPC)
                nc.sync.dma_start(s[:], b1_d[e])
                b1_t.append(s)
            b2_t = cpool.tile([EPC, D], f32)
            nc.sync.dma_start(b2_t[:], b2_d[:])
            lnw_t = cpool.tile([B, D], f32)
            nc.sync.dma_start(lnw_t[:], lnw_d[:])
            lnb_t = cpool.tile([B, D], f32)
            nc.sync.dma_start(lnb_t[:], lnb_d[:])
            wlos_t = cpool.tile([128, DC * OUT], f32)
            for dc in range(DC):
                nc.sync.dma_start(
                    wlos_t[:, dc * OUT:(dc + 1) * OUT],
                    wlos_d[dc * 128:(dc + 1) * 128, :])
            blos_t = cpool.tile([OUT, 1], f32)
            nc.sync.dma_start(blos_t[:], blos_d[:])
            ty_t = cpool.tile([B, OUT], f32)
            nc.sync.dma_start(ty_t[:], ty_d[:])
            ones = cpool.tile([128, 1], f32)
            nc.vector.memset(ones[:], 1.0)

            # ---- expert weights (streamed; Tile prefetches into free slots) ----
            w1_t = [[None] * DC for _ in range(EPC)]
            w2_t = [[None] * HC for _ in range(EPC)]
            for e in range(EPC):
                for dc in range(DC):
                    t = w1pool.tile([128, H], f32r, tag="w1")
                    nc.sync.dma_start(t[:], w1_d[e, dc * 128:(dc + 1) * 128, :])
                    w1_t[e][dc] = t
                for hc in range(HC):
                    t = w2pool.tile([128, D], f32r, tag="w2")
                    nc.sync.dma_start(t[:], w2_d[e, hc * 128:(hc + 1) * 128, :])
                    w2_t[e][hc] = t

            # ---- gating: logitsT = wg.T @ x (fp32), transpose, top-4 softmax ----
            logits_t = []
            for g in range(NG):
                lgt_ps = smps.tile([E, TG], f32, tag="sp")
                for dc in range(DC):
                    nc.tensor.matmul(
                        lgt_ps[:],
                        wg_t[:, dc * E:(dc + 1) * E],
                        xtr[dc][:, g * TG:(g + 1) * TG].bitcast(f32),
                        start=(dc == 0), stop=(dc == DC - 1))
                lgt_sb = spool.tile([E, TG], f32, tag="lgt")
                nc.scalar.copy(lgt_sb[:], lgt_ps[:])
                for c8 in range(TG // 128):
                    tp = smps.tile([128, E], f32, tag="sp")
                    nc.tensor.transpose(
                        tp[:], lgt_sb[:, c8 * 128:(c8 + 1) * 128], ident[:E, :E])
                    lt = spool.tile([128, E], f32, tag="logits", bufs=8)
                    nc.scalar.copy(lt[:], tp[:])
                    logits_t.append(lt)

            gates_t, mask_t, gcol = [], [], []
            for t_ in range(TC):
                lt = logits_t[t_]
                mx8 = spool.tile([128, 8], f32, tag="mx8", bufs=2)
                nc.vector.max(mx8[:], lt[:])
                negm = spool.tile([128, 1], f32, tag="negm", bufs=2)
                nc.vector.reduce_max(negm[:], lt[:], axis=AX.X, negate=True)
                mask = spool.tile([128, E], f32, tag="mask", bufs=8)
                nc.vector.tensor_scalar(
                    mask[:], lt[:], mx8[:, 3:4], None, op0=mybir.AluOpType.is_ge)
                el = spool.tile([128, E], f32, tag="el", bufs=2)
                nc.scalar.activation(el[:], lt[:], AF.Exp, bias=negm[:, 0:1])
                gated = spool.tile([128, E], f32, tag="gated", bufs=2)
                nc.vector.tensor_mul(gated[:], el[:], mask[:])
                denom = spool.tile([128, 1], f32, tag="denom", bufs=2)
                nc.vector.reduce_sum(denom[:], gated[:], axis=AX.X)
                rden = spool.tile([128, 1], f32, tag="rden", bufs=2)
                nc.vector.reciprocal(rden[:], denom[:])
                gates = spool.tile([128, E], f32, tag="gates", bufs=8)
                nc.vector.tensor_scalar_mul(gates[:], gated[:], rden[:, 0:1])
                gates_t.append(gates)
                mask_t.append(mask)
                # per-expert gate columns for this core (data-driven select)
                cols = []
                for e in range(EPC):
                    gm = spool.tile([128, E], f32, tag="gm", bufs=2)
                    nc.vector.tensor_mul(gm[:], gates[:], esel_t[e][:])
                    gc = spool.tile([128, 1], f32, tag="gcol", bufs=16)
                    nc.vector.reduce_sum(gc[:], gm[:], axis=AX.X)
                    cols.append(gc)
                gcol.append(cols)

            # ---- aux loss: importance / load ----
            def colsum16(tiles, tag):
                ps = smps.tile([1, E], f32, tag="sp")
                for t_ in range(TC):
                    nc.tensor.matmul(ps[:], ones[:], tiles[t_][:],
                                     start=(t_ == 0), stop=(t_ == TC - 1))
                sb = spool.tile([1, E], f32, tag=tag)
                nc.scalar.copy(sb[:], ps[:])
                return sb

            imp_sb = colsum16(gates_t, "imp")
            load_sb = colsum16(mask_t, "load")

            def cv_parts(v16, tag):
                # returns (sum_sq_dev [1,1], recip_mean2e [1,1]); cv = ssd/15 * r
                s = spool.tile([1, 1], f32, tag=tag + "s")
                nc.vector.reduce_sum(s[:], v16[:], axis=AX.X)
                mean = spool.tile([1, 1], f32, tag=tag + "m")
                nc.vector.tensor_scalar_mul(mean[:], s[:], 1.0 / E)
                d = spool.tile([1, E], f32, tag=tag + "d")
                nc.vector.tensor_scalar(
                    d[:], v16[:], mean[0:1, 0:1], None,
                    op0=mybir.AluOpType.subtract)
                d2 = spool.tile([1, E], f32, tag=tag + "d2")
                nc.vector.tensor_mul(d2[:], d[:], d[:])
                ssd = spool.tile([1, 1], f32, tag=tag + "v")
                nc.vector.reduce_sum(ssd[:], d2[:], axis=AX.X)
                m2 = spool.tile([1, 1], f32, tag=tag + "m2")
                nc.vector.tensor_mul(m2[:], mean[:], mean[:])
                m2e = spool.tile([1, 1], f32, tag=tag + "m2e")
                nc.vector.tensor_scalar_add(m2e[:], m2[:], 1e-10)
                r = spool.tile([1, 1], f32, tag=tag + "r")
                nc.vector.reciprocal(r[:], m2e[:])
                cv = spool.tile([1, 1], f32, tag=tag + "cv")
                nc.vector.tensor_mul(cv[:], ssd[:], r[:])
                return cv

            cvi = cv_parts(imp_sb, "ci")
            cvl = cv_parts(load_sb, "cl")
            cvs = spool.tile([1, 1], f32, tag="cvs")
            nc.vector.tensor_add(cvs[:], cvi[:], cvl[:])
            loss_sb = spool.tile([1, 1], f32, tag="lossv")
            nc.vector.tensor_scalar_mul(loss_sb[:], cvs[:], 0.01 / (E - 1))
            nc.sync.dma_start(loss_d[:], loss_sb[:])

            # ---- gb = S.T @ gates (per-batch gate sums), then this core's rows ----
            gb_ps = smps.tile([B, E], f32, tag="sp")
            for t_ in range(TC):
                nc.tensor.matmul(gb_ps[:], smat_t[t_][:], gates_t[t_][:],
                                 start=(t_ == 0), stop=(t_ == TC - 1))
            gb_sb = spool.tile([B, E], f32, tag="gb")
            nc.scalar.copy(gb_sb[:], gb_ps[:])
            gbt_ps = smps.tile([E, B], f32, tag="sp")
            nc.tensor.transpose(gbt_ps[:], gb_sb[:], ident[:B, :B])
            gbt_sb = spool.tile([E, B], f32, tag="gbt")
            nc.scalar.copy(gbt_sb[:], gbt_ps[:])
            gbt2_ps = smps.tile([EPC, B], f32, tag="sp")
            nc.tensor.matmul(gbt2_ps[:], eselt_t[:], gbt_sb[:], start=True, stop=True)
            gbt2_sb = spool.tile([EPC, B], f32, tag="gbt2")
            nc.scalar.copy(gbt2_sb[:], gbt2_ps[:])

            # ---- expert MLPs + gated combine + modality sum (into mm psum) ----
            SKIP = os.environ.get("KSKIP", "")
            mm_ps = [mmps.tile([B, TG], f32, tag="mm", name=f"mm_ps{i}")
                     for i in range(2)]
            first_mm = [True, True]
            for e in range(0 if "experts" in SKIP else EPC):
                for g in range(NG):
                    hT = []
                    for hc in range(HC):
                        ph = bigps.tile([128, TG], f32, tag="big")
                        for dc in range(DC):
                            nc.tensor.matmul(
                                ph[:],
                                w1_t[e][dc][:, hc * 128:(hc + 1) * 128],
                                xtr[dc][:, g * TG:(g + 1) * TG],
                                start=(dc == 0), stop=(dc == DC - 1))
                        ht = hpool.tile([128, TG], f32r, tag="hT")
                        nc.scalar.activation(ht[:], ph[:], AF.Relu,
                                             bias=b1_t[e][:, hc:hc + 1])
                        hT.append(ht)
                    for sub in range(TG // 128):
                        tglob = g * (TG // 128) + sub
                        ge = spool.tile([128, B], f32r, tag="ge", bufs=3)
                        nc.vector.tensor_scalar_mul(
                            ge[:], smat_t[tglob][:], gcol[tglob][e][:, 0:1])
                        for dh in range(2):
                            po = bigps.tile([128, TG], f32, tag="big")
                            for hc in range(HC):
                                nc.tensor.matmul(
                                    po[:],
                                    hT[hc][:, sub * 128:(sub + 1) * 128],
                                    w2_t[e][hc][:, dh * TG:(dh + 1) * TG],
                                    start=(hc == 0), stop=(hc == HC - 1))
                            osb = opool.tile([128, TG], f32r, tag="osb")
                            nc.scalar.copy(osb[:], po[:])
                            nc.tensor.matmul(mm_ps[dh][:], ge[:], osb[:],
                                             start=first_mm[dh], stop=False)
                            first_mm[dh] = False
            # b2 contribution: gbt2.T @ b2c rows (K=2), closes the mm groups
            for dh in range(2):
                nc.tensor.matmul(mm_ps[dh][:], gbt2_sb[:],
                                 b2_t[:, dh * TG:(dh + 1) * TG].bitcast(f32),
                                 start=("experts" in SKIP), stop=True)

            # ---- AllReduce partial mm across expert groups ----
            mm_sb = spool.tile([B, D], f32, tag="lnbuf", bufs=3)
            for dh in range(2):
                nc.scalar.copy(mm_sb[:, dh * TG:(dh + 1) * TG], mm_ps[dh][:])
            in_b = dpool.tile([B, D], f32)
            out_b = dpool.tile([B, D], f32, addr_space="Shared")
            nc.sync.dma_start(in_b[:], mm_sb[:])
            if skip_cc:
                nc.sync.dma_start(out_b[:], in_b[:])
            else:
                nc.gpsimd.collective_compute(
                    "AllReduce", mybir.AluOpType.add,
                    replica_groups=[list(range(n_cores))],
                    ins=[in_b.opt()], outs=[out_b.opt()])
            mmr = spool.tile([B, D], f32, tag="lnbuf", bufs=3)
            nc.sync.dma_start(mmr[:], out_b[:])

            # ---- LayerNorm over D (biased var = E[x^2] - E[x]^2) ----
            sq = spool.tile([B, D], f32, tag="lnbuf", bufs=3)
            s2 = spool.tile([B, 1], f32, tag="s2")
            nc.scalar.activation(sq[:], mmr[:], AF.Square, accum_out=s2[:, 0:1])
            s1 = spool.tile([B, 1], f32, tag="s1")
            nc.vector.reduce_sum(s1[:], mmr[:], axis=AX.X)
            negmu = spool.tile([B, 1], f32, tag="negmu")
            nc.vector.tensor_scalar_mul(negmu[:], s1[:], -1.0 / D)
            mu2 = spool.tile([B, 1], f32, tag="mu2")
            nc.vector.tensor_mul(mu2[:], negmu[:], negmu[:])
            epsb = spool.tile([B, 1], f32, tag="epsb")
            nc.vector.memset(epsb[:], 1e-5)
            # var + eps = s2/D - mu^2 + eps  via tensor_scalar(s2*(1/D) - mu2) + eps
            ve = spool.tile([B, 1], f32, tag="ve")
            nc.vector.tensor_scalar(
                ve[:], s2[:], 1.0 / D, mu2[0:B, 0:1],
                op0=mybir.AluOpType.mult, op1=mybir.AluOpType.subtract)
            std = spool.tile([B, 1], f32, tag="std")
            nc.scalar.activation(std[:], ve[:], AF.Sqrt, bias=epsb[:, 0:1])
            rstd = spool.tile([B, 1], f32, tag="rstd")
            nc.vector.reciprocal(rstd[:], std[:])
            # (x - mu) * rstd in one pass, then affine
            xn = spool.tile([B, D], f32, tag="lnbuf", bufs=3)
            nc.vector.tensor_scalar(
                xn[:], mmr[:], negmu[0:B, 0:1], rstd[0:B, 0:1],
                op0=mybir.AluOpType.add, op1=mybir.AluOpType.mult)
            xw = spool.tile([B, D], f32, tag="lnbuf", bufs=3)
            nc.vector.tensor_mul(xw[:], xn[:], lnw_t[:])
            fin = spool.tile([B, D], f32, tag="lnbuf", bufs=3)
            nc.vector.tensor_add(fin[:], xw[:], lnb_t[:])

            # ---- LOS head: scoresT = wlos.T @ finT, + b_los ----
            st_ps = mmps.tile([OUT, B], f32, tag="mm")
            for dc in range(DC):
                tp = bigps.tile([128, B], f32, tag="big")
                nc.tensor.transpose(
                    tp[:], fin[:, dc * 128:(dc + 1) * 128], ident[:B, :B])
                ft = spool.tile([128, B], f32, tag="fT", bufs=4)
                nc.scalar.copy(ft[:], tp[:])
                nc.tensor.matmul(st_ps[:], wlos_t[:, dc * OUT:(dc + 1) * OUT],
                                 ft[:], start=(dc == 0), stop=(dc == DC - 1))
            st_sb = spool.tile([OUT, B], f32, tag="stsb")
            nc.scalar.add(st_sb[:], st_ps[:], blos_t[:, 0:1])
            sc_ps = mmps.tile([B, OUT], f32, tag="mm")
            nc.tensor.transpose(sc_ps[:], st_sb[:], ident[:OUT, :OUT])
            sc_sb = spool.tile([B, OUT], f32, tag="scsb")
            nc.scalar.copy(sc_sb[:], sc_ps[:])
            nc.sync.dma_start(scores_d[:], sc_sb[:])

            # ---- pred_loss = mean((scores - y)^2) ----
            df = spool.tile([B, OUT], f32, tag="df")
            nc.vector.tensor_sub(df[:], sc_sb[:], ty_t[:])
            dfs = spool.tile([B, OUT], f32, tag="dfs")
            nc.vector.tensor_mul(dfs[:], df[:], df[:])
            rs = spool.tile([B, 1], f32, tag="rs")
            nc.vector.reduce_sum(rs[:], dfs[:], axis=AX.X)
            pl_ps = bigps.tile([1, 1], f32, tag="big")
            nc.tensor.matmul(pl_ps[:], ones[:B, :], rs[:], start=True, stop=True)
            pl_sb = spool.tile([1, 1], f32, tag="plsb")
            nc.scalar.mul(pl_sb[:], pl_ps[:], 1.0 / (B * OUT))
            nc.sync.dma_start(ploss_d[:], pl_sb[:])

    nc.compile()
    return nc


def _host_inputs(inputs):
    f = np.float32
    x = np.asarray(inputs["mm_embed"], f).reshape(N, D)
    xT = np.ascontiguousarray(x.T)
    wg = np.asarray(inputs["w_gate"], f)
    W1 = np.asarray(inputs["W1"], f)
    b1 = np.asarray(inputs["b1"], f)
    W2 = np.asarray(inputs["W2"], f)
    b2 = np.asarray(inputs["b2"], f)
    lnw_b = np.broadcast_to(np.asarray(inputs["ln_w"], f), (B, D)).copy()
    lnb_b = np.broadcast_to(np.asarray(inputs["ln_b"], f), (B, D)).copy()
    wlos = np.asarray(inputs["W_los"], f)
    blos = np.asarray(inputs["b_los"], f).reshape(OUT, 1)
    ty = np.asarray(inputs["true_y"], f)
    smat = np.zeros((N, B), f)
    smat[np.arange(N), np.arange(N) // M] = 1.0
    ident = np.eye(128, dtype=f)

    in_maps = []
    for c in range(N_CORES):
        es = np.zeros((EPC, 128, E), f)
        est = np.zeros((E, EPC), f)
        for e in range(EPC):
            es[e, :, c * EPC + e] = 1.0
            est[c * EPC + e, e] = 1.0
        b1r = np.ascontiguousarray(
            b1[c * EPC:(c + 1) * EPC].reshape(EPC, HC, 128).transpose(0, 2, 1))
        in_maps.append({
            "xtr": xT,
            "wg": wg,
            "w1c": np.ascontiguousarray(W1[c * EPC:(c + 1) * EPC]),
            "w2c": np.ascontiguousarray(W2[c * EPC:(c + 1) * EPC]),
            "b1r": b1r,
            "b2c": np.ascontiguousarray(b2[c * EPC:(c + 1) * EPC]),
            "smat": smat,
            "esel": es,
            "eselt": est,
            "lnw_b": lnw_b,
            "lnb_b": lnb_b,
            "wlos": wlos,
            "blos": blos,
            "ty": ty,
            "ident": ident,
        })
    return in_maps


def get_nc():
    if "nc" not in _CACHE:
        _CACHE["nc"] = _build()
    return _CACHE["nc"]


def kernel(**inputs):
    nc = get_nc()
    in_maps = _host_inputs(inputs)
    res = bass_utils.run_bass_kernel_spmd(nc, in_maps, core_ids=list(range(N_CORES)))
    r0 = res.results[0]
    scores = np.asarray(r0["scores"], np.float32)
    loss = np.asarray(r0["loss"], np.float32).reshape(())
    ploss = np.asarray(r0["ploss"], np.float32).reshape(())
    return (scores, loss, ploss)


if __name__ == "__main__":
    import reference
    inputs = {k: np.asarray(v) if not np.isscalar(v) else v
              for k, v in reference.setup_inputs().items()}
    got = kernel(**inputs)
    exp = reference.reference(**reference.setup_inputs())
    for name, g_, e_ in zip(("scores", "loss", "pred_loss"), got, exp):
        e_ = np.asarray(e_)
        rel = np.abs(g_ - e_).max() / (np.abs(e_).max() + 1e-12)
        print(f"{name}: rel err {rel:.3e}")


# revision 14
# speedup vs baseline: 1.1233x; 1.0834x over previous
"""MoE routing kernel for Trainium2, 8-core expert-parallel.

Strategy: each core owns 2 of 16 experts (expert-parallel, per the sharding
hint). Gating (fp32 matmul + top-4 softmax) is replicated on every core —
it is tiny — so the load-balance loss needs no collective. Each core runs
its two experts' MLPs densely over all 1024 tokens with float32r matmuls
(full-rate fp32 storage, TF32-ish compute), folds the gate-weighted combine
and the modality sum into PE matmuls, then an AllReduce of the [64, 1024]
partial combines expert groups. LayerNorm + LOS head + both aux losses are
replicated on-device; the host just shards inputs and reads core 0's output.
"""

import os

import numpy as np

import concourse.mybir as mybir
import concourse.tile as tile
from concourse import bacc
from concourse import bass_utils

N_CORES = 8
E, K = 16, 4
D, H = 1024, 1024
B, M = 64, 16
N = B * M          # 1024 tokens
OUT = 10
TG = 512           # token group (matmul moving free dim)
NG = N // TG       # 2 token groups
DC = D // 128      # 8
HC = H // 128      # 8
TC = N // 128      # 8 token tiles
EPC = E // N_CORES  # 2 experts per core

_CACHE: dict = {}


def _build(n_cores=N_CORES, skip_cc=False):
    dt = mybir.dt
    nc = bacc.Bacc("TRN2", target_bir_lowering=False, debug=False,
                   num_devices=n_cores)

    # ---- DRAM I/O ----
    xtr_d = nc.dram_tensor("xtr", [D, N], dt.float32r, kind="ExternalInput").ap()
    wg_d = nc.dram_tensor("wg", [D, E], dt.float32, kind="ExternalInput").ap()
    w1_d = nc.dram_tensor("w1c", [EPC, D, H], dt.float32r, kind="ExternalInput").ap()
    w2_d = nc.dram_tensor("w2c", [EPC, H, D], dt.float32r, kind="ExternalInput").ap()
    b1_d = nc.dram_tensor("b1r", [EPC, 128, HC], dt.float32, kind="ExternalInput").ap()
    b2_d = nc.dram_tensor("b2c", [EPC, D], dt.float32, kind="ExternalInput").ap()
    smat_d = nc.dram_tensor("smat", [N, B], dt.float32, kind="ExternalInput").ap()
    esel_d = nc.dram_tensor("esel", [EPC, 128, E], dt.float32, kind="ExternalInput").ap()
    eselt_d = nc.dram_tensor("eselt", [E, EPC], dt.float32, kind="ExternalInput").ap()
    wlos_d = nc.dram_tensor("wlos", [D, OUT], dt.float32, kind="ExternalInput").ap()
    wsum_d = nc.dram_tensor("wsum_b", [B, OUT], dt.float32, kind="ExternalInput").ap()
    blos2_d = nc.dram_tensor("blos2_b", [B, OUT], dt.float32, kind="ExternalInput").ap()
    ty_d = nc.dram_tensor("ty", [B, OUT], dt.float32, kind="ExternalInput").ap()
    ident_d = nc.dram_tensor("ident", [128, 128], dt.float32, kind="ExternalInput").ap()

    scores_d = nc.dram_tensor("scores", [B, OUT], dt.float32, kind="ExternalOutput").ap()
    loss_d = nc.dram_tensor("loss", [1, 1], dt.float32, kind="ExternalOutput").ap()
    ploss_d = nc.dram_tensor("ploss", [1, 1], dt.float32, kind="ExternalOutput").ap()

    f32, f32r = dt.float32, dt.float32r
    AF = mybir.ActivationFunctionType
    AX = mybir.AxisListType

    with tile.TileContext(nc) as tc:
        with tc.tile_pool(name="const", bufs=1) as cpool, \
             tc.tile_pool(name="xtr", bufs=DC) as xpool, \
             tc.tile_pool(name="w1", bufs=12) as w1pool, \
             tc.tile_pool(name="w2", bufs=12) as w2pool, \
             tc.tile_pool(name="hT", bufs=9) as hpool, \
             tc.tile_pool(name="osb", bufs=3) as opool, \
             tc.tile_pool(name="sm", bufs=1) as spool, \
             tc.tile_pool(name="mm", bufs=2, space="PSUM") as mmps, \
             tc.tile_pool(name="big", bufs=3, space="PSUM") as bigps, \
             tc.tile_pool(name="small", bufs=2, space="PSUM") as smps, \
             tc.tile_pool(name="dram", bufs=2, space="DRAM") as dpool:

            # ---- consts on the gpsimd DMA queue (parallel to bulk loads) ----
            wg_t = cpool.tile([128, DC * E], f32)
            nc.gpsimd.dma_start(
                wg_t[:].rearrange("p (c e) -> p c e", c=DC),
                wg_d.rearrange("(c p) e -> p c e", p=128))
            ident = cpool.tile([128, 128], f32)
            nc.gpsimd.dma_start(ident[:], ident_d[:])
            xtr = []
            for dc in range(DC):
                t = xpool.tile([128, N], f32r)
                nc.sync.dma_start(t[:], xtr_d[dc * 128:(dc + 1) * 128, :])
                xtr.append(t)
            smat_all = cpool.tile([128, TC * B], f32)
            nc.gpsimd.dma_start(
                smat_all[:].rearrange("p (c b) -> p c b", c=TC),
                smat_d.rearrange("(c p) b -> p c b", p=128))
            smat_t = [smat_all[:, t_ * B:(t_ + 1) * B] for t_ in range(TC)]
            esel_all = cpool.tile([128, EPC * E], f32)
            nc.gpsimd.dma_start(
                esel_all[:].rearrange("p (e j) -> p e j", e=EPC),
                esel_d.rearrange("e p j -> p e j"))
            esel_t = [esel_all[:, e * E:(e + 1) * E] for e in range(EPC)]
            eselt_t = cpool.tile([E, EPC], f32)
            nc.gpsimd.dma_start(eselt_t[:], eselt_d[:])
            b1_all = cpool.tile([128, EPC * HC], f32)
            nc.gpsimd.dma_start(
                b1_all[:].rearrange("p (e h) -> p e h", e=EPC),
                b1_d.rearrange("e p h -> p e h"))
            b1_t = [b1_all[:, e * HC:(e + 1) * HC] for e in range(EPC)]
            b2_t = cpool.tile([EPC, D], f32)
            nc.gpsimd.dma_start(b2_t[:], b2_d[:])
            wlos_t = cpool.tile([128, DC * OUT], f32)
            nc.gpsimd.dma_start(
                wlos_t[:].rearrange("p (c o) -> p c o", c=DC),
                wlos_d.rearrange("(c p) o -> p c o", p=128))
            wsum_t = cpool.tile([B, OUT], f32)
            nc.gpsimd.dma_start(wsum_t[:], wsum_d[:])
            blos2_t = cpool.tile([B, OUT], f32)
            nc.gpsimd.dma_start(blos2_t[:], blos2_d[:])
            ty_t = cpool.tile([B, OUT], f32)
            nc.gpsimd.dma_start(ty_t[:], ty_d[:])
            ones = cpool.tile([128, 1], f32)
            nc.vector.memset(ones[:], 1.0)
            # pre-warm ACT tables used in the serial tail
            warm = spool.tile([1, 1], f32, tag="warm")
            nc.vector.memset(warm[:], 1.0)
            warm2 = spool.tile([1, 1], f32, tag="warm2")
            nc.scalar.activation(warm2[:], warm[:], AF.Square)
            warm3 = spool.tile([1, 1], f32, tag="warm3")
            nc.scalar.activation(warm3[:], warm[:], AF.Sqrt)

            # ---- expert weights (streamed; Tile prefetches into free slots) ----
            w1_t = [[None] * DC for _ in range(EPC)]
            w2_t = [[None] * HC for _ in range(EPC)]
            for e in range(EPC):
                for dc in range(DC):
                    t = w1pool.tile([128, H], f32r, tag="w1")
                    nc.sync.dma_start(t[:], w1_d[e, dc * 128:(dc + 1) * 128, :])
                    w1_t[e][dc] = t
                for hc in range(HC):
                    t = w2pool.tile([128, D], f32r, tag="w2")
                    nc.sync.dma_start(t[:], w2_d[e, hc * 128:(hc + 1) * 128, :])
                    w2_t[e][hc] = t

            # ---- gating: logitsT = wg.T @ x (fp32), transpose, top-4 softmax ----
            logits_t = []
            for g in range(NG):
                lgt_ps = smps.tile([E, TG], f32, tag="sp")
                for dc in range(DC):
                    nc.tensor.matmul(
                        lgt_ps[:],
                        wg_t[:, dc * E:(dc + 1) * E],
                        xtr[dc][:, g * TG:(g + 1) * TG].bitcast(f32),
                        start=(dc == 0), stop=(dc == DC - 1))
                lgt_sb = spool.tile([E, TG], f32, tag="lgt")
                nc.scalar.copy(lgt_sb[:], lgt_ps[:])
                for c8 in range(TG // 128):
                    tp = smps.tile([128, E], f32, tag="sp")
                    nc.tensor.transpose(
                        tp[:], lgt_sb[:, c8 * 128:(c8 + 1) * 128], ident[:E, :E])
                    lt = spool.tile([128, E], f32, tag="logits", bufs=8)
                    nc.scalar.copy(lt[:], tp[:])
                    logits_t.append(lt)

            gates_t, mask_t, gcol = [], [], []
            for t_ in range(TC):
                lt = logits_t[t_]
                mx8 = spool.tile([128, 8], f32, tag="mx8", bufs=2)
                nc.vector.max(mx8[:], lt[:])
                negm = spool.tile([128, 1], f32, tag="negm", bufs=2)
                nc.vector.reduce_max(negm[:], lt[:], axis=AX.X, negate=True)
                mask = spool.tile([128, E], f32, tag="mask", bufs=8)
                nc.vector.tensor_scalar(
                    mask[:], lt[:], mx8[:, 3:4], None, op0=mybir.AluOpType.is_ge)
                el = spool.tile([128, E], f32, tag="el", bufs=2)
                nc.scalar.activation(el[:], lt[:], AF.Exp, bias=negm[:, 0:1])
                gated = spool.tile([128, E], f32, tag="gated", bufs=2)
                nc.vector.tensor_mul(gated[:], el[:], mask[:])
                denom = spool.tile([128, 1], f32, tag="denom", bufs=2)
                nc.vector.reduce_sum(denom[:], gated[:], axis=AX.X)
                rden = spool.tile([128, 1], f32, tag="rden", bufs=2)
                nc.vector.reciprocal(rden[:], denom[:])
                gates = spool.tile([128, E], f32, tag="gates", bufs=8)
                nc.vector.tensor_scalar_mul(gates[:], gated[:], rden[:, 0:1])
                gates_t.append(gates)
                mask_t.append(mask)
                # per-expert gate columns for this core (data-driven select)
                cols = []
                for e in range(EPC):
                    gm = spool.tile([128, E], f32, tag="gm", bufs=2)
                    nc.vector.tensor_mul(gm[:], gates[:], esel_t[e])
                    gc = spool.tile([128, 1], f32, tag="gcol", bufs=16)
                    nc.vector.reduce_sum(gc[:], gm[:], axis=AX.X)
                    cols.append(gc)
                gcol.append(cols)

            # ---- aux loss: importance / load ----
            def colsum16(tiles, tag):
                ps = smps.tile([1, E], f32, tag="sp")
                for t_ in range(TC):
                    nc.tensor.matmul(ps[:], ones[:], tiles[t_][:],
                                     start=(t_ == 0), stop=(t_ == TC - 1))
                sb = spool.tile([1, E], f32, tag=tag)
                nc.scalar.copy(sb[:], ps[:])
                return sb

            imp_sb = colsum16(gates_t, "imp")
            load_sb = colsum16(mask_t, "load")

            def cv_parts(v16, tag):
                # returns (sum_sq_dev [1,1], recip_mean2e [1,1]); cv = ssd/15 * r
                s = spool.tile([1, 1], f32, tag=tag + "s")
                nc.vector.reduce_sum(s[:], v16[:], axis=AX.X)
                mean = spool.tile([1, 1], f32, tag=tag + "m")
                nc.vector.tensor_scalar_mul(mean[:], s[:], 1.0 / E)
                d = spool.tile([1, E], f32, tag=tag + "d")
                nc.vector.tensor_scalar(
                    d[:], v16[:], mean[0:1, 0:1], None,
                    op0=mybir.AluOpType.subtract)
                d2 = spool.tile([1, E], f32, tag=tag + "d2")
                nc.vector.tensor_mul(d2[:], d[:], d[:])
                ssd = spool.tile([1, 1], f32, tag=tag + "v")
                nc.vector.reduce_sum(ssd[:], d2[:], axis=AX.X)
                m2 = spool.tile([1, 1], f32, tag=tag + "m2")
                nc.vector.tensor_mul(m2[:], mean[:], mean[:])
                m2e = spool.tile([1, 1], f32, tag=tag + "m2e")
                nc.vector.tensor_scalar_add(m2e[:], m2[:], 1e-10)
                r = spool.tile([1, 1], f32, tag=tag + "r")
                nc.vector.reciprocal(r[:], m2e[:])
                cv = spool.tile([1, 1], f32, tag=tag + "cv")
                nc.vector.tensor_mul(cv[:], ssd[:], r[:])
                return cv

            cvi = cv_parts(imp_sb, "ci")
            cvl = cv_parts(load_sb, "cl")
            cvs = spool.tile([1, 1], f32, tag="cvs")
            nc.vector.tensor_add(cvs[:], cvi[:], cvl[:])
            loss_sb = spool.tile([1, 1], f32, tag="lossv")
            nc.vector.tensor_scalar_mul(loss_sb[:], cvs[:], 0.01 / (E - 1))
            nc.sync.dma_start(loss_d[:], loss_sb[:])

            # ---- gb = S.T @ gates (per-batch gate sums), then this core's rows ----
            gb_ps = smps.tile([B, E], f32, tag="sp")
            for t_ in range(TC):
                nc.tensor.matmul(gb_ps[:], smat_t[t_], gates_t[t_][:],
                                 start=(t_ == 0), stop=(t_ == TC - 1))
            gb_sb = spool.tile([B, E], f32, tag="gb")
            nc.scalar.copy(gb_sb[:], gb_ps[:])
            gbt_ps = smps.tile([E, B], f32, tag="sp")
            nc.tensor.transpose(gbt_ps[:], gb_sb[:], ident[:B, :B])
            gbt_sb = spool.tile([E, B], f32, tag="gbt")
            nc.scalar.copy(gbt_sb[:], gbt_ps[:])
            gbt2_ps = smps.tile([EPC, B], f32, tag="sp")
            nc.tensor.matmul(gbt2_ps[:], eselt_t[:], gbt_sb[:], start=True, stop=True)
            gbt2_sb = spool.tile([EPC, B], f32, tag="gbt2")
            nc.scalar.copy(gbt2_sb[:], gbt2_ps[:])

            # ---- expert MLPs + gated combine + modality sum (into mm psum) ----
            SKIP = os.environ.get("KSKIP", "")
            mm_ps = [mmps.tile([B, TG], f32, tag="mm", name=f"mm_ps{i}")
                     for i in range(2)]
            # b2 contribution opens each accumulation group
            for dh in range(2):
                nc.tensor.matmul(mm_ps[dh][:], gbt2_sb[:],
                                 b2_t[:, dh * TG:(dh + 1) * TG].bitcast(f32),
                                 start=True, stop=False)
            n_ge = [0, 0]
            for e in range(0 if "experts" in SKIP else EPC):
                for g in range(NG):
                    hT = []
                    for hc in range(HC):
                        ph = bigps.tile([128, TG], f32, tag="big")
                        for dc in range(DC):
                            nc.tensor.matmul(
                                ph[:],
                                w1_t[e][dc][:, hc * 128:(hc + 1) * 128],
                                xtr[dc][:, g * TG:(g + 1) * TG],
                                start=(dc == 0), stop=(dc == DC - 1))
                        ht = hpool.tile([128, TG], f32r, tag="hT")
                        nc.scalar.activation(ht[:], ph[:], AF.Relu,
                                             bias=b1_t[e][:, hc:hc + 1])
                        hT.append(ht)
                    for sub in range(TG // 128):
                        tglob = g * (TG // 128) + sub
                        ge = spool.tile([128, B], f32r, tag="ge", bufs=3)
                        nc.vector.tensor_scalar_mul(
                            ge[:], smat_t[tglob], gcol[tglob][e][:, 0:1])
                        for dh in range(2):
                            po = bigps.tile([128, TG], f32, tag="big")
                            for hc in range(HC):
                                nc.tensor.matmul(
                                    po[:],
                                    hT[hc][:, sub * 128:(sub + 1) * 128],
                                    w2_t[e][hc][:, dh * TG:(dh + 1) * TG],
                                    start=(hc == 0), stop=(hc == HC - 1))
                            osb = opool.tile([128, TG], f32r, tag="osb")
                            nc.scalar.copy(osb[:], po[:])
                            n_ge[dh] += 1
                            nc.tensor.matmul(mm_ps[dh][:], ge[:], osb[:],
                                             start=False,
                                             stop=(n_ge[dh] == EPC * NG * 4))
            # ---- AllReduce partial mm across expert groups ----
            mm_sb = spool.tile([B, D], f32, tag="lnbuf", bufs=3)
            for dh in range(2):
                nc.scalar.copy(mm_sb[:, dh * TG:(dh + 1) * TG], mm_ps[dh][:])
            in_b = dpool.tile([B, D], f32)
            out_b = dpool.tile([B, D], f32, addr_space="Shared")
            nc.sync.dma_start(in_b[:], mm_sb[:])
            if skip_cc:
                nc.sync.dma_start(out_b[:], in_b[:])
            else:
                nc.gpsimd.collective_compute(
                    "AllReduce", mybir.AluOpType.add,
                    replica_groups=[list(range(n_cores))],
                    ins=[in_b.opt()], outs=[out_b.opt()])
            mmr = spool.tile([B, D], f32, tag="lnbuf", bufs=3)
            nc.sync.dma_start(mmr[:], out_b[:])

            # ---- LN stats (mu, rstd) — runs concurrent with the head matmuls ----
            sq = spool.tile([B, D], f32, tag="lnbuf", bufs=3)
            s2 = spool.tile([B, 1], f32, tag="s2")
            nc.scalar.activation(sq[:], mmr[:], AF.Square, accum_out=s2[:, 0:1])
            s1 = spool.tile([B, 1], f32, tag="s1")
            nc.vector.reduce_sum(s1[:], mmr[:], axis=AX.X)
            negmu = spool.tile([B, 1], f32, tag="negmu")
            nc.vector.tensor_scalar_mul(negmu[:], s1[:], -1.0 / D)
            mu2 = spool.tile([B, 1], f32, tag="mu2")
            nc.vector.tensor_mul(mu2[:], negmu[:], negmu[:])
            epsb = spool.tile([B, 1], f32, tag="epsb")
            nc.vector.memset(epsb[:], 1e-5)
            ve = spool.tile([B, 1], f32, tag="ve")
            nc.vector.tensor_scalar(
                ve[:], s2[:], 1.0 / D, mu2[0:B, 0:1],
                op0=mybir.AluOpType.mult, op1=mybir.AluOpType.subtract)
            std = spool.tile([B, 1], f32, tag="std")
            nc.scalar.activation(std[:], ve[:], AF.Sqrt, bias=epsb[:, 0:1])
            rstd = spool.tile([B, 1], f32, tag="rstd")
            nc.vector.reciprocal(rstd[:], std[:])

            # ---- head on raw mmr: scores = rstd*(mmr@wlos') - mu*rstd*wsum + blos' ----
            st_ps = mmps.tile([OUT, B], f32, tag="mm")
            for dc in range(DC):
                tp = bigps.tile([128, B], f32, tag="big")
                nc.tensor.transpose(
                    tp[:], mmr[:, dc * 128:(dc + 1) * 128], ident[:B, :B])
                ft = spool.tile([128, B], f32, tag="fT", bufs=4)
                nc.scalar.copy(ft[:], tp[:])
                nc.tensor.matmul(st_ps[:], wlos_t[:, dc * OUT:(dc + 1) * OUT],
                                 ft[:], start=(dc == 0), stop=(dc == DC - 1))
            st_sb = spool.tile([OUT, B], f32, tag="stsb")
            nc.scalar.copy(st_sb[:], st_ps[:])
            sc_ps = mmps.tile([B, OUT], f32, tag="mm")
            nc.tensor.transpose(sc_ps[:], st_sb[:], ident[:OUT, :OUT])
            mneg = spool.tile([B, 1], f32, tag="mneg")
            nc.vector.tensor_mul(mneg[:], negmu[:], rstd[:])
            term2 = spool.tile([B, OUT], f32, tag="term2")
            nc.vector.tensor_scalar_mul(term2[:], wsum_t[:], mneg[:, 0:1])
            sc1 = spool.tile([B, OUT], f32, tag="sc1")
            nc.vector.tensor_scalar_mul(sc1[:], sc_ps[:], rstd[:, 0:1])
            sc2 = spool.tile([B, OUT], f32, tag="sc2")
            nc.vector.tensor_add(sc2[:], sc1[:], term2[:])
            sc_sb = spool.tile([B, OUT], f32, tag="scsb")
            nc.vector.tensor_add(sc_sb[:], sc2[:], blos2_t[:])
            nc.sync.dma_start(scores_d[:], sc_sb[:])

            # ---- pred_loss = mean((scores - y)^2) ----
            df = spool.tile([B, OUT], f32, tag="df")
            nc.vector.tensor_sub(df[:], sc_sb[:], ty_t[:])
            dfs = spool.tile([B, OUT], f32, tag="dfs")
            nc.vector.tensor_mul(dfs[:], df[:], df[:])
            rs = spool.tile([B, 1], f32, tag="rs")
            nc.vector.reduce_sum(rs[:], dfs[:], axis=AX.X)
            pl_ps = bigps.tile([1, 1], f32, tag="big")
            nc.tensor.matmul(pl_ps[:], ones[:B, :], rs[:], start=True, stop=True)
            pl_sb = spool.tile([1, 1], f32, tag="plsb")
            nc.scalar.mul(pl_sb[:], pl_ps[:], 1.0 / (B * OUT))
            nc.sync.dma_start(ploss_d[:], pl_sb[:])

    nc.compile()
    return nc


def _host_inputs(inputs):
    f = np.float32
    x = np.asarray(inputs["mm_embed"], f).reshape(N, D)
    xT = np.ascontiguousarray(x.T)
    wg = np.asarray(inputs["w_gate"], f)
    W1 = np.asarray(inputs["W1"], f)
    b1 = np.asarray(inputs["b1"], f)
    W2 = np.asarray(inputs["W2"], f)
    b2 = np.asarray(inputs["b2"], f)
    lnw = np.asarray(inputs["ln_w"], f)
    lnb = np.asarray(inputs["ln_b"], f)
    wlos_raw = np.asarray(inputs["W_los"], f)
    blos_raw = np.asarray(inputs["b_los"], f)
    wlos = np.ascontiguousarray(lnw[:, None] * wlos_raw)
    wsum_b = np.broadcast_to(wlos.sum(0), (B, OUT)).astype(f).copy()
    blos2_b = np.broadcast_to(blos_raw + lnb @ wlos_raw, (B, OUT)).astype(f).copy()
    ty = np.asarray(inputs["true_y"], f)
    smat = np.zeros((N, B), f)
    smat[np.arange(N), np.arange(N) // M] = 1.0
    ident = np.eye(128, dtype=f)

    in_maps = []
    for c in range(N_CORES):
        es = np.zeros((EPC, 128, E), f)
        est = np.zeros((E, EPC), f)
        for e in range(EPC):
            es[e, :, c * EPC + e] = 1.0
            est[c * EPC + e, e] = 1.0
        b1r = np.ascontiguousarray(
            b1[c * EPC:(c + 1) * EPC].reshape(EPC, HC, 128).transpose(0, 2, 1))
        in_maps.append({
            "xtr": xT,
            "wg": wg,
            "w1c": np.ascontiguousarray(W1[c * EPC:(c + 1) * EPC]),
            "w2c": np.ascontiguousarray(W2[c * EPC:(c + 1) * EPC]),
            "b1r": b1r,
            "b2c": np.ascontiguousarray(b2[c * EPC:(c + 1) * EPC]),
            "smat": smat,
            "esel": es,
            "eselt": est,
            "wlos": wlos,
            "wsum_b": wsum_b,
            "blos2_b": blos2_b,
            "ty": ty,
            "ident": ident,
        })
    return in_maps


def get_nc():
    if "nc" not in _CACHE:
        _CACHE["nc"] = _build()
    return _CACHE["nc"]


def kernel(**inputs):
    nc = get_nc()
    in_maps = _host_inputs(inputs)
    res = bass_utils.run_bass_kernel_spmd(nc, in_maps, core_ids=list(range(N_CORES)))
    r0 = res.results[0]
    scores = np.asarray(r0["scores"], np.float32)
    loss = np.asarray(r0["loss"], np.float32).reshape(())
    ploss = np.asarray(r0["ploss"], np.float32).reshape(())
    return (scores, loss, ploss)


if __name__ == "__main__":
    import reference
    inputs = {k: np.asarray(v) if not np.isscalar(v) else v
              for k, v in reference.setup_inputs().items()}
    got = kernel(**inputs)
    exp = reference.reference(**reference.setup_inputs())
    for name, g_, e_ in zip(("scores", "loss", "pred_loss"), got, exp):
        e_ = np.asarray(e_)
        rel = np.abs(g_ - e_).max() / (np.abs(e_).max() + 1e-12)
        print(f"{name}: rel err {rel:.3e}")


# revision 15
# speedup vs baseline: 1.1414x; 1.0161x over previous
"""MoE routing kernel for Trainium2, 8-core expert-parallel.

Strategy: each core owns 2 of 16 experts (expert-parallel, per the sharding
hint). Gating (fp32 matmul + top-4 softmax) is replicated on every core —
it is tiny — so the load-balance loss needs no collective. Each core runs
its two experts' MLPs densely over all 1024 tokens with float32r matmuls
(full-rate fp32 storage, TF32-ish compute), folds the gate-weighted combine
and the modality sum into PE matmuls, then an AllReduce of the [64, 1024]
partial combines expert groups. LayerNorm + LOS head + both aux losses are
replicated on-device; the host just shards inputs and reads core 0's output.
"""

import os

import numpy as np

import concourse.mybir as mybir
import concourse.tile as tile
from concourse import bacc
from concourse import bass_utils

N_CORES = 8
E, K = 16, 4
D, H = 1024, 1024
B, M = 64, 16
N = B * M          # 1024 tokens
OUT = 10
TG = 512           # token group (matmul moving free dim)
NG = N // TG       # 2 token groups
DC = D // 128      # 8
HC = H // 128      # 8
TC = N // 128      # 8 token tiles
EPC = E // N_CORES  # 2 experts per core

_CACHE: dict = {}


def _build(n_cores=N_CORES, skip_cc=False):
    dt = mybir.dt
    nc = bacc.Bacc("TRN2", target_bir_lowering=False, debug=False,
                   num_devices=n_cores)

    # ---- DRAM I/O ----
    xtr_d = nc.dram_tensor("xtr", [D, N], dt.float32r, kind="ExternalInput").ap()
    wg_d = nc.dram_tensor("wg", [D, E], dt.float32, kind="ExternalInput").ap()
    w1_d = nc.dram_tensor("w1c", [EPC, D, H], dt.float32r, kind="ExternalInput").ap()
    w2_d = nc.dram_tensor("w2c", [EPC, H, D], dt.float32r, kind="ExternalInput").ap()
    b1_d = nc.dram_tensor("b1r", [EPC, 128, HC], dt.float32, kind="ExternalInput").ap()
    b2_d = nc.dram_tensor("b2c", [EPC, D], dt.float32, kind="ExternalInput").ap()
    smat_d = nc.dram_tensor("smat", [N, B], dt.float32, kind="ExternalInput").ap()
    esel_d = nc.dram_tensor("esel", [EPC, 128, E], dt.float32, kind="ExternalInput").ap()
    eselt_d = nc.dram_tensor("eselt", [E, EPC], dt.float32, kind="ExternalInput").ap()
    wlos_d = nc.dram_tensor("wlos", [D, OUT], dt.float32, kind="ExternalInput").ap()
    wsum_d = nc.dram_tensor("wsum_b", [B, OUT], dt.float32, kind="ExternalInput").ap()
    blos2_d = nc.dram_tensor("blos2_b", [B, OUT], dt.float32, kind="ExternalInput").ap()
    ty_d = nc.dram_tensor("ty", [B, OUT], dt.float32, kind="ExternalInput").ap()
    ident_d = nc.dram_tensor("ident", [128, 128], dt.float32, kind="ExternalInput").ap()

    scores_d = nc.dram_tensor("scores", [B, OUT], dt.float32, kind="ExternalOutput").ap()
    loss_d = nc.dram_tensor("loss", [1, 1], dt.float32, kind="ExternalOutput").ap()
    ploss_d = nc.dram_tensor("ploss", [1, 1], dt.float32, kind="ExternalOutput").ap()

    f32, f32r = dt.float32, dt.float32r
    AF = mybir.ActivationFunctionType
    AX = mybir.AxisListType

    with tile.TileContext(nc) as tc:
        with tc.tile_pool(name="const", bufs=1) as cpool, \
             tc.tile_pool(name="xtr", bufs=DC) as xpool, \
             tc.tile_pool(name="w1", bufs=12) as w1pool, \
             tc.tile_pool(name="w2", bufs=12) as w2pool, \
             tc.tile_pool(name="hT", bufs=9) as hpool, \
             tc.tile_pool(name="osb", bufs=3) as opool, \
             tc.tile_pool(name="sm", bufs=1) as spool, \
             tc.tile_pool(name="mm", bufs=2, space="PSUM") as mmps, \
             tc.tile_pool(name="big", bufs=3, space="PSUM") as bigps, \
             tc.tile_pool(name="small", bufs=2, space="PSUM") as smps, \
             tc.tile_pool(name="dram", bufs=2, space="DRAM") as dpool:

            # ---- consts on the gpsimd DMA queue (parallel to bulk loads) ----
            wg_t = cpool.tile([128, DC * E], f32)
            nc.sync.dma_start(
                wg_t[:].rearrange("p (c e) -> p c e", c=DC),
                wg_d.rearrange("(c p) e -> p c e", p=128))
            ident = cpool.tile([128, 128], f32)
            nc.gpsimd.dma_start(ident[:], ident_d[:])
            xtr = []
            for dc in range(DC):
                t = xpool.tile([128, N], f32r)
                nc.sync.dma_start(t[:], xtr_d[dc * 128:(dc + 1) * 128, :])
                xtr.append(t)
            smat_all = cpool.tile([128, TC * B], f32)
            nc.gpsimd.dma_start(
                smat_all[:].rearrange("p (c b) -> p c b", c=TC),
                smat_d.rearrange("(c p) b -> p c b", p=128))
            smat_t = [smat_all[:, t_ * B:(t_ + 1) * B] for t_ in range(TC)]
            esel_all = cpool.tile([128, EPC * E], f32)
            nc.gpsimd.dma_start(
                esel_all[:].rearrange("p (e j) -> p e j", e=EPC),
                esel_d.rearrange("e p j -> p e j"))
            esel_t = [esel_all[:, e * E:(e + 1) * E] for e in range(EPC)]
            eselt_t = cpool.tile([E, EPC], f32)
            nc.gpsimd.dma_start(eselt_t[:], eselt_d[:])
            b1_all = cpool.tile([128, EPC * HC], f32)
            nc.gpsimd.dma_start(
                b1_all[:].rearrange("p (e h) -> p e h", e=EPC),
                b1_d.rearrange("e p h -> p e h"))
            b1_t = [b1_all[:, e * HC:(e + 1) * HC] for e in range(EPC)]
            b2_t = cpool.tile([EPC, D], f32)
            nc.gpsimd.dma_start(b2_t[:], b2_d[:])
            wlos_t = cpool.tile([128, DC * OUT], f32)
            nc.gpsimd.dma_start(
                wlos_t[:].rearrange("p (c o) -> p c o", c=DC),
                wlos_d.rearrange("(c p) o -> p c o", p=128))
            wsum_t = cpool.tile([B, OUT], f32)
            nc.gpsimd.dma_start(wsum_t[:], wsum_d[:])
            blos2_t = cpool.tile([B, OUT], f32)
            nc.gpsimd.dma_start(blos2_t[:], blos2_d[:])
            ty_t = cpool.tile([B, OUT], f32)
            nc.gpsimd.dma_start(ty_t[:], ty_d[:])
            ones = cpool.tile([128, 1], f32)
            nc.vector.memset(ones[:], 1.0)
            # pre-warm ACT tables used in the serial tail
            warm = spool.tile([1, 1], f32, tag="warm")
            nc.vector.memset(warm[:], 1.0)
            warm2 = spool.tile([1, 1], f32, tag="warm2")
            nc.scalar.activation(warm2[:], warm[:], AF.Square)
            warm3 = spool.tile([1, 1], f32, tag="warm3")
            nc.scalar.activation(warm3[:], warm[:], AF.Sqrt)

            # ---- expert weights (streamed; Tile prefetches into free slots) ----
            w1_t = [[None] * DC for _ in range(EPC)]
            w2_t = [[None] * HC for _ in range(EPC)]
            for e in range(EPC):
                for dc in range(DC):
                    t = w1pool.tile([128, H], f32r, tag="w1")
                    nc.sync.dma_start(t[:], w1_d[e, dc * 128:(dc + 1) * 128, :])
                    w1_t[e][dc] = t
                for hc in range(HC):
                    t = w2pool.tile([128, D], f32r, tag="w2")
                    nc.sync.dma_start(t[:], w2_d[e, hc * 128:(hc + 1) * 128, :])
                    w2_t[e][hc] = t

            # ---- gating: logitsT = wg.T @ x (fp32), transpose, top-4 softmax ----
            logits_t = []
            for g in range(NG):
                lgt_ps = smps.tile([E, TG], f32, tag="sp")
                for dc in range(DC):
                    nc.tensor.matmul(
                        lgt_ps[:],
                        wg_t[:, dc * E:(dc + 1) * E],
                        xtr[dc][:, g * TG:(g + 1) * TG].bitcast(f32),
                        start=(dc == 0), stop=(dc == DC - 1))
                lgt_sb = spool.tile([E, TG], f32, tag="lgt")
                nc.scalar.copy(lgt_sb[:], lgt_ps[:])
                for c8 in range(TG // 128):
                    tp = smps.tile([128, E], f32, tag="sp")
                    nc.tensor.transpose(
                        tp[:], lgt_sb[:, c8 * 128:(c8 + 1) * 128], ident[:E, :E])
                    lt = spool.tile([128, E], f32, tag="logits", bufs=8)
                    nc.scalar.copy(lt[:], tp[:])
                    logits_t.append(lt)

            gates_t, mask_t, gcol = [], [], []
            for t_ in range(TC):
                lt = logits_t[t_]
                mx8 = spool.tile([128, 8], f32, tag="mx8", bufs=2)
                nc.vector.max(mx8[:], lt[:])
                negm = spool.tile([128, 1], f32, tag="negm", bufs=2)
                nc.vector.reduce_max(negm[:], lt[:], axis=AX.X, negate=True)
                mask = spool.tile([128, E], f32, tag="mask", bufs=8)
                nc.vector.tensor_scalar(
                    mask[:], lt[:], mx8[:, 3:4], None, op0=mybir.AluOpType.is_ge)
                el = spool.tile([128, E], f32, tag="el", bufs=2)
                nc.scalar.activation(el[:], lt[:], AF.Exp, bias=negm[:, 0:1])
                gated = spool.tile([128, E], f32, tag="gated", bufs=2)
                nc.vector.tensor_mul(gated[:], el[:], mask[:])
                denom = spool.tile([128, 1], f32, tag="denom", bufs=2)
                nc.vector.reduce_sum(denom[:], gated[:], axis=AX.X)
                rden = spool.tile([128, 1], f32, tag="rden", bufs=2)
                nc.vector.reciprocal(rden[:], denom[:])
                gates = spool.tile([128, E], f32, tag="gates", bufs=8)
                nc.vector.tensor_scalar_mul(gates[:], gated[:], rden[:, 0:1])
                gates_t.append(gates)
                mask_t.append(mask)
                # per-expert gate columns for this core (data-driven select)
                cols = []
                for e in range(EPC):
                    gm = spool.tile([128, E], f32, tag="gm", bufs=2)
                    nc.vector.tensor_mul(gm[:], gates[:], esel_t[e])
                    gc = spool.tile([128, 1], f32, tag="gcol", bufs=16)
                    nc.vector.reduce_sum(gc[:], gm[:], axis=AX.X)
                    cols.append(gc)
                gcol.append(cols)

            # ---- aux loss: importance / load ----
            def colsum16(tiles, tag):
                ps = smps.tile([1, E], f32, tag="sp")
                for t_ in range(TC):
                    nc.tensor.matmul(ps[:], ones[:], tiles[t_][:],
                                     start=(t_ == 0), stop=(t_ == TC - 1))
                sb = spool.tile([1, E], f32, tag=tag)
                nc.scalar.copy(sb[:], ps[:])
                return sb

            imp_sb = colsum16(gates_t, "imp")
            load_sb = colsum16(mask_t, "load")

            def cv_parts(v16, tag):
                # returns (sum_sq_dev [1,1], recip_mean2e [1,1]); cv = ssd/15 * r
                s = spool.tile([1, 1], f32, tag=tag + "s")
                nc.vector.reduce_sum(s[:], v16[:], axis=AX.X)
                mean = spool.tile([1, 1], f32, tag=tag + "m")
                nc.vector.tensor_scalar_mul(mean[:], s[:], 1.0 / E)
                d = spool.tile([1, E], f32, tag=tag + "d")
                nc.vector.tensor_scalar(
                    d[:], v16[:], mean[0:1, 0:1], None,
                    op0=mybir.AluOpType.subtract)
                d2 = spool.tile([1, E], f32, tag=tag + "d2")
                nc.vector.tensor_mul(d2[:], d[:], d[:])
                ssd = spool.tile([1, 1], f32, tag=tag + "v")
                nc.vector.reduce_sum(ssd[:], d2[:], axis=AX.X)
                m2 = spool.tile([1, 1], f32, tag=tag + "m2")
                nc.vector.tensor_mul(m2[:], mean[:], mean[:])
                m2e = spool.tile([1, 1], f32, tag=tag + "m2e")
                nc.vector.tensor_scalar_add(m2e[:], m2[:], 1e-10)
                r = spool.tile([1, 1], f32, tag=tag + "r")
                nc.vector.reciprocal(r[:], m2e[:])
                cv = spool.tile([1, 1], f32, tag=tag + "cv")
                nc.vector.tensor_mul(cv[:], ssd[:], r[:])
                return cv

            cvi = cv_parts(imp_sb, "ci")
            cvl = cv_parts(load_sb, "cl")
            cvs = spool.tile([1, 1], f32, tag="cvs")
            nc.vector.tensor_add(cvs[:], cvi[:], cvl[:])
            loss_sb = spool.tile([1, 1], f32, tag="lossv")
            nc.vector.tensor_scalar_mul(loss_sb[:], cvs[:], 0.01 / (E - 1))
            nc.sync.dma_start(loss_d[:], loss_sb[:])

            # ---- gb = S.T @ gates (per-batch gate sums), then this core's rows ----
            gb_ps = smps.tile([B, E], f32, tag="sp")
            for t_ in range(TC):
                nc.tensor.matmul(gb_ps[:], smat_t[t_], gates_t[t_][:],
                                 start=(t_ == 0), stop=(t_ == TC - 1))
            gb_sb = spool.tile([B, E], f32, tag="gb")
            nc.scalar.copy(gb_sb[:], gb_ps[:])
            gbt_ps = smps.tile([E, B], f32, tag="sp")
            nc.tensor.transpose(gbt_ps[:], gb_sb[:], ident[:B, :B])
            gbt_sb = spool.tile([E, B], f32, tag="gbt")
            nc.scalar.copy(gbt_sb[:], gbt_ps[:])
            gbt2_ps = smps.tile([EPC, B], f32, tag="sp")
            nc.tensor.matmul(gbt2_ps[:], eselt_t[:], gbt_sb[:], start=True, stop=True)
            gbt2_sb = spool.tile([EPC, B], f32, tag="gbt2")
            nc.scalar.copy(gbt2_sb[:], gbt2_ps[:])

            # ---- expert MLPs + gated combine + modality sum (into mm psum) ----
            SKIP = os.environ.get("KSKIP", "")
            mm_ps = [mmps.tile([B, TG], f32, tag="mm", name=f"mm_ps{i}")
                     for i in range(2)]
            # b2 contribution opens each accumulation group
            for dh in range(2):
                nc.tensor.matmul(mm_ps[dh][:], gbt2_sb[:],
                                 b2_t[:, dh * TG:(dh + 1) * TG].bitcast(f32),
                                 start=True, stop=False)
            n_ge = [0, 0]
            for e in range(0 if "experts" in SKIP else EPC):
                for g in range(NG):
                    hT = []
                    for hc in range(HC):
                        ph = bigps.tile([128, TG], f32, tag="big")
                        for dc in range(DC):
                            nc.tensor.matmul(
                                ph[:],
                                w1_t[e][dc][:, hc * 128:(hc + 1) * 128],
                                xtr[dc][:, g * TG:(g + 1) * TG],
                                start=(dc == 0), stop=(dc == DC - 1))
                        ht = hpool.tile([128, TG], f32r, tag="hT")
                        nc.scalar.activation(ht[:], ph[:], AF.Relu,
                                             bias=b1_t[e][:, hc:hc + 1])
                        hT.append(ht)
                    for sub in range(TG // 128):
                        tglob = g * (TG // 128) + sub
                        smr = spool.tile([128, B], f32r, tag="smr", bufs=8)
                        nc.vector.tensor_copy(smr[:], smat_t[tglob])
                        for dh in range(2):
                            po = bigps.tile([128, TG], f32, tag="big")
                            for hc in range(HC):
                                nc.tensor.matmul(
                                    po[:],
                                    hT[hc][:, sub * 128:(sub + 1) * 128],
                                    w2_t[e][hc][:, dh * TG:(dh + 1) * TG],
                                    start=(hc == 0), stop=(hc == HC - 1))
                            osb = opool.tile([128, TG], f32r, tag="osb")
                            nc.scalar.mul(osb[:], po[:], gcol[tglob][e][:, 0:1])
                            n_ge[dh] += 1
                            nc.tensor.matmul(mm_ps[dh][:], smr[:], osb[:],
                                             start=False,
                                             stop=(n_ge[dh] == EPC * NG * 4))
            # ---- AllReduce partial mm, two pipelined halves ----
            mm_sb = spool.tile([B, D], f32, tag="lnbuf", bufs=3)
            mmr = spool.tile([B, D], f32, tag="lnbuf", bufs=3)
            for dh in range(2):
                sl = slice(dh * TG, (dh + 1) * TG)
                nc.scalar.copy(mm_sb[:, sl], mm_ps[dh][:])
                in_b = dpool.tile([B, TG], f32, tag="inb", bufs=2)
                out_b = dpool.tile([B, TG], f32, addr_space="Shared",
                                   tag="outb", bufs=2)
                nc.sync.dma_start(in_b[:], mm_sb[:, sl])
                if skip_cc:
                    nc.sync.dma_start(out_b[:], in_b[:])
                else:
                    nc.gpsimd.collective_compute(
                        "AllReduce", mybir.AluOpType.add,
                        replica_groups=[list(range(n_cores))],
                        ins=[in_b.opt()], outs=[out_b.opt()])
                nc.sync.dma_start(mmr[:, sl], out_b[:])

            # ---- LN stats (mu, rstd) — runs concurrent with the head matmuls ----
            sq = spool.tile([B, D], f32, tag="lnbuf", bufs=3)
            s2 = spool.tile([B, 1], f32, tag="s2")
            nc.scalar.activation(sq[:], mmr[:], AF.Square, accum_out=s2[:, 0:1])
            s1 = spool.tile([B, 1], f32, tag="s1")
            nc.vector.reduce_sum(s1[:], mmr[:], axis=AX.X)
            negmu = spool.tile([B, 1], f32, tag="negmu")
            nc.vector.tensor_scalar_mul(negmu[:], s1[:], -1.0 / D)
            mu2 = spool.tile([B, 1], f32, tag="mu2")
            nc.vector.tensor_mul(mu2[:], negmu[:], negmu[:])
            epsb = spool.tile([B, 1], f32, tag="epsb")
            nc.vector.memset(epsb[:], 1e-5)
            ve = spool.tile([B, 1], f32, tag="ve")
            nc.vector.tensor_scalar(
                ve[:], s2[:], 1.0 / D, mu2[0:B, 0:1],
                op0=mybir.AluOpType.mult, op1=mybir.AluOpType.subtract)
            std = spool.tile([B, 1], f32, tag="std")
            nc.scalar.activation(std[:], ve[:], AF.Sqrt, bias=epsb[:, 0:1])
            rstd = spool.tile([B, 1], f32, tag="rstd")
            nc.vector.reciprocal(rstd[:], std[:])

            # ---- head on raw mmr: scores = rstd*(mmr@wlos') - mu*rstd*wsum + blos' ----
            st_ps = mmps.tile([OUT, B], f32, tag="mm")
            for dc in range(DC):
                tp = bigps.tile([128, B], f32, tag="big")
                nc.tensor.transpose(
                    tp[:], mmr[:, dc * 128:(dc + 1) * 128], ident[:B, :B])
                ft = spool.tile([128, B], f32, tag="fT", bufs=4)
                nc.scalar.copy(ft[:], tp[:])
                nc.tensor.matmul(st_ps[:], wlos_t[:, dc * OUT:(dc + 1) * OUT],
                                 ft[:], start=(dc == 0), stop=(dc == DC - 1))
            st_sb = spool.tile([OUT, B], f32, tag="stsb")
            nc.scalar.copy(st_sb[:], st_ps[:])
            sc_ps = mmps.tile([B, OUT], f32, tag="mm")
            nc.tensor.transpose(sc_ps[:], st_sb[:], ident[:OUT, :OUT])
            mneg = spool.tile([B, 1], f32, tag="mneg")
            nc.vector.tensor_mul(mneg[:], negmu[:], rstd[:])
            term2 = spool.tile([B, OUT], f32, tag="term2")
            nc.vector.tensor_scalar_mul(term2[:], wsum_t[:], mneg[:, 0:1])
            sc1 = spool.tile([B, OUT], f32, tag="sc1")
            nc.vector.tensor_scalar_mul(sc1[:], sc_ps[:], rstd[:, 0:1])
            sc2 = spool.tile([B, OUT], f32, tag="sc2")
            nc.vector.tensor_add(sc2[:], sc1[:], term2[:])
            sc_sb = spool.tile([B, OUT], f32, tag="scsb")
            nc.vector.tensor_add(sc_sb[:], sc2[:], blos2_t[:])
            nc.sync.dma_start(scores_d[:], sc_sb[:])

            # ---- pred_loss = mean((scores - y)^2) ----
            df = spool.tile([B, OUT], f32, tag="df")
            nc.vector.tensor_sub(df[:], sc_sb[:], ty_t[:])
            dfs = spool.tile([B, OUT], f32, tag="dfs")
            nc.vector.tensor_mul(dfs[:], df[:], df[:])
            rs = spool.tile([B, 1], f32, tag="rs")
            nc.vector.reduce_sum(rs[:], dfs[:], axis=AX.X)
            pl_ps = bigps.tile([1, 1], f32, tag="big")
            nc.tensor.matmul(pl_ps[:], ones[:B, :], rs[:], start=True, stop=True)
            pl_sb = spool.tile([1, 1], f32, tag="plsb")
            nc.scalar.mul(pl_sb[:], pl_ps[:], 1.0 / (B * OUT))
            nc.sync.dma_start(ploss_d[:], pl_sb[:])

    nc.compile()
    return nc


def _host_inputs(inputs):
    f = np.float32
    x = np.asarray(inputs["mm_embed"], f).reshape(N, D)
    xT = np.ascontiguousarray(x.T)
    wg = np.asarray(inputs["w_gate"], f)
    W1 = np.asarray(inputs["W1"], f)
    b1 = np.asarray(inputs["b1"], f)
    W2 = np.asarray(inputs["W2"], f)
    b2 = np.asarray(inputs["b2"], f)
    lnw = np.asarray(inputs["ln_w"], f)
    lnb = np.asarray(inputs["ln_b"], f)
    wlos_raw = np.asarray(inputs["W_los"], f)
    blos_raw = np.asarray(inputs["b_los"], f)
    wlos = np.ascontiguousarray(lnw[:, None] * wlos_raw)
    wsum_b = np.broadcast_to(wlos.sum(0), (B, OUT)).astype(f).copy()
    blos2_b = np.broadcast_to(blos_raw + lnb @ wlos_raw, (B, OUT)).astype(f).copy()
    ty = np.asarray(inputs["true_y"], f)
    smat = np.zeros((N, B), f)
    smat[np.arange(N), np.arange(N) // M] = 1.0
    ident = np.eye(128, dtype=f)

    in_maps = []
    for c in range(N_CORES):
        es = np.zeros((EPC, 128, E), f)
        est = np.zeros((E, EPC), f)
        for e in range(EPC):
            es[e, :, c * EPC + e] = 1.0
            est[c * EPC + e, e] = 1.0
        b1r = np.ascontiguousarray(
            b1[c * EPC:(c + 1) * EPC].reshape(EPC, HC, 128).transpose(0, 2, 1))
        in_maps.append({
            "xtr": xT,
            "wg": wg,
            "w1c": np.ascontiguousarray(W1[c * EPC:(c + 1) * EPC]),
            "w2c": np.ascontiguousarray(W2[c * EPC:(c + 1) * EPC]),
            "b1r": b1r,
            "b2c": np.ascontiguousarray(b2[c * EPC:(c + 1) * EPC]),
            "smat": smat,
            "esel": es,
            "eselt": est,
            "wlos": wlos,
            "wsum_b": wsum_b,
            "blos2_b": blos2_b,
            "ty": ty,
            "ident": ident,
        })
    return in_maps


def get_nc():
    if "nc" not in _CACHE:
        _CACHE["nc"] = _build()
    return _CACHE["nc"]


def kernel(**inputs):
    nc = get_nc()
    in_maps = _host_inputs(inputs)
    res = bass_utils.run_bass_kernel_spmd(nc, in_maps, core_ids=list(range(N_CORES)))
    r0 = res.results[0]
    scores = np.asarray(r0["scores"], np.float32)
    loss = np.asarray(r0["loss"], np.float32).reshape(())
    ploss = np.asarray(r0["ploss"], np.float32).reshape(())
    return (scores, loss, ploss)


if __name__ == "__main__":
    import reference
    inputs = {k: np.asarray(v) if not np.isscalar(v) else v
              for k, v in reference.setup_inputs().items()}
    got = kernel(**inputs)
    exp = reference.reference(**reference.setup_inputs())
    for name, g_, e_ in zip(("scores", "loss", "pred_loss"), got, exp):
        e_ = np.asarray(e_)
        rel = np.abs(g_ - e_).max() / (np.abs(e_).max() + 1e-12)
        print(f"{name}: rel err {rel:.3e}")
